# revision 27
# baseline (speedup 1.0000x reference)
"""Trainium2 Bass kernel for nn_KSpaceLoss: exact type-2 NUFFT k-space loss.

loss = 0.1 * (sum|d| / sum|a|) + 0.1 * sqrt(sum d^2 / sum a^2)
  d = (E @ x) * mask - kdata * mask,  a = kdata * mask
  E[k, n] = exp(-2j*pi * traj[:, k] . r[:, n])   (K=8192, N=96*96)

Strategy:
  * K axis: masked-out samples contribute 0 to both sums -> gather kept
    columns on host, pad to KP = 8*KL, shard over 8 cores (KL each).
  * Phase factorization: n=(nx,ny), nx=32*nx1+nx0, ny=48*ny1+ny0 gives
    E[n,k] = U[g,k] * V[m,k] with g=(nx1,ny1) in 6 groups and
    m=(nx0,ny0) in 1536 members. V ([1536,K] twiddle table, ~16% of E)
    and U ([6,K]) are host-precomputed; V is shipped as fp8e4.
  * ksp[k,c] = sum_g U[g,k] * W_g[k,c],  W_g = V^T @ x_g  as fp8e4
    DoubleRow matmuls (2 m-chunks per matmul, 0.5 cyc/row). Weights pack
    both groups of a pair into 128 columns [xr_e|xi_e|xr_o|xi_o]: stream
    Vr against that, Vi against [-xi_e|xr_e|-xi_o|xr_o], accumulating
    [Wre_e|Wim_e|Wre_o|Wim_o] in PSUM over all 12 member-chunks.
  * U applied per pair with two elementwise products (W * U-pack -> bf16)
    on DVE; each product is folded straight into kr/ki PSUM accumulators
    with +-1 sign matmuls on the PE (emitted one pair behind the DR
    stream so the PE never stalls). Zero-padding of the gathered K axis
    is folded into the U-packs (U=0 there -> ksp=0, kdata=0 -> d=0).
  * Residual: combined [64,KL] d = [kr;ki] - [kdr;kdi], one fused
    square+reduce (sum d^2), a ones-fold matmul for dr^2+di^2, and one
    Sqrt activation with accumulation (sum |d|). |a| sums are computed
    on host (O(K), input-only). Host does the final scalar combine.
"""

import math

import numpy as np

import concourse.bacc as bacc
import concourse.tile as tile
from concourse import mybir
from concourse.bass_utils import run_bass_kernel_spmd

X, Y = 96, 96
C, T = 8, 4
K = 8192
N = X * Y
NCORES = 8
CST = C * T               # 32
G = 6                     # groups: nx1 in [0,3), ny1 in [0,2)
M = 1536                  # members: nx0 in [0,32), ny0 in [0,48)
MCH = M // 128            # 12 member chunks
CP = MCH // 2             # 6 DoubleRow chunk-pairs
NPAIR = G // 2            # 3 group pairs
W1, W2 = 0.1, 0.1

F32 = mybir.dt.float32
F16 = mybir.dt.float16
BF16 = mybir.dt.bfloat16
F8 = mybir.dt.float8e4

KL_PRIMARY = 640          # per-core columns; covers mask count <= 5120
KL_FULL = 1024            # fallback: all 8192 columns fit


def _bank_slices(kl):
    out, j = [], 0
    while j < kl:
        je = min(j + 512, kl)
        out.append((j, je))
        j = je
    return out


def build_kernel(KL):
    nc = bacc.Bacc("TRN2", target_bir_lowering=False, debug=False,
                   num_devices=NCORES)

    w1_d = nc.dram_tensor("w1", [NPAIR, 128, CP, 2, 128], F8,
                          kind="ExternalInput").ap()
    w2_d = nc.dram_tensor("w2", [NPAIR, 128, CP, 2, 128], F8,
                          kind="ExternalInput").ap()
    vr_d = nc.dram_tensor("vr", [CP, 128, 2, KL], F8, kind="ExternalInput").ap()
    vi_d = nc.dram_tensor("vi", [CP, 128, 2, KL], F8, kind="ExternalInput").ap()
    ua_d = nc.dram_tensor("ua", [NPAIR, 128, KL], F16, kind="ExternalInput").ap()
    ub_d = nc.dram_tensor("ub", [NPAIR, 128, KL], F16, kind="ExternalInput").ap()
    kdri_d = nc.dram_tensor("kdri", [2 * CST, KL], F32, kind="ExternalInput").ap()
    sgn_d = nc.dram_tensor("sgn", [128, 3, CST], BF16, kind="ExternalInput").ap()
    parts_d = nc.dram_tensor("parts", [2 * CST, 2], F32, kind="ExternalOutput").ap()

    Sqrt = mybir.ActivationFunctionType.Sqrt
    Alu = mybir.AluOpType
    DR = mybir.MatmulPerfMode.DoubleRow
    JS = _bank_slices(KL)

    DMA_SPLIT = 4
    INTERLEAVE_FOLDS = False

    def dma4(dst, src):
        # split a [128, ...] load into partition slices -> parallel queues
        if DMA_SPLIT == 1:
            nc.sync.dma_start(dst[:], src[:])
            return
        step = 128 // DMA_SPLIT
        for q in range(DMA_SPLIT):
            sl = slice(step * q, step * (q + 1))
            nc.sync.dma_start(dst[sl], src[sl])

    with tile.TileContext(nc) as tc:
        with (
            tc.tile_pool(name="const", bufs=1) as cpool,
            tc.tile_pool(name="wacc", bufs=2, space="PSUM") as wpool,
            tc.tile_pool(name="fin", bufs=1, space="PSUM") as fpool,
            tc.tile_pool(name="prod", bufs=3) as prp,
            tc.tile_pool(name="resid", bufs=1) as rsp,
        ):
            # ---- constant loads, priority-ordered and queue-spread ----
            w1t = [cpool.tile([128, CP, 2, 128], F8, tag=f"w1_{p}", name=f"w1t{p}")
                   for p in range(NPAIR)]
            w2t = [cpool.tile([128, CP, 2, 128], F8, tag=f"w2_{p}", name=f"w2t{p}")
                   for p in range(NPAIR)]
            vr8 = cpool.tile([128, CP, 2, KL], F8, tag="vr8")
            vi8 = cpool.tile([128, CP, 2, KL], F8, tag="vi8")
            uat = [cpool.tile([128, KL], F16, tag=f"ua_{p}", name=f"uat{p}")
                   for p in range(NPAIR)]
            ubt = [cpool.tile([128, KL], F16, tag=f"ub_{p}", name=f"ubt{p}")
                   for p in range(NPAIR)]
            sgn = cpool.tile([128, 3, CST], BF16, tag="sgn")
            kdri = cpool.tile([2 * CST, KL], F32, tag="kdri")

            dma4(w1t[0][:], w1_d[0])
            dma4(vr8[:, 0], vr_d[0])
            dma4(w2t[0][:], w2_d[0])
            dma4(vi8[:, 0], vi_d[0])
            for cp in range(1, CP):
                dma4(vr8[:, cp], vr_d[cp])
                dma4(vi8[:, cp], vi_d[cp])
            nc.sync.dma_start(sgn[:], sgn_d[:])
            for p in range(1, NPAIR):
                dma4(w1t[p][:], w1_d[p])
                dma4(w2t[p][:], w2_d[p])
            for p in range(NPAIR):
                dma4(uat[p][:], ua_d[p])
                dma4(ubt[p][:], ub_d[p])
            nc.sync.dma_start(kdri[:], kdri_d[:])

            parts = rsp.tile([2 * CST, 2], F32, tag="parts")
            nc.vector.memset(parts[:], 0.0)

            # ---- per pair: PSUM W accumulation -> U products -> folds ----
            kri = fpool.tile([2 * CST, 1024], F32, tag="kri")
            prods = []          # (p1, p2) awaiting fold
            fold_p = [0]

            def emit_folds():
                p = fold_p[0]
                p1, p2 = prods[p]
                for (js, je) in JS:
                    nc.tensor.matmul(kri[0:CST, js:je], sgn[:, 0, :],
                                     p1[:, js:je],
                                     start=(p == 0), stop=(p == NPAIR - 1))
                for (js, je) in JS:
                    nc.tensor.matmul(kri[CST:2 * CST, js:je], sgn[:, 1, :],
                                     p2[:, js:je],
                                     start=(p == 0), stop=(p == NPAIR - 1))
                fold_p[0] += 1

            for p in range(NPAIR):
                W = wpool.tile([128, 1024], F32, tag="W")
                for cp in range(CP):
                    for (js, je) in JS:
                        nc.tensor.matmul(W[:, js:je], w1t[p][:, cp],
                                         vr8[:, cp, :, js:je],
                                         perf_mode=DR,
                                         start=(cp == 0), stop=False)
                    for (js, je) in JS:
                        nc.tensor.matmul(W[:, js:je], w2t[p][:, cp],
                                         vi8[:, cp, :, js:je],
                                         perf_mode=DR,
                                         start=False, stop=(cp == CP - 1))
                    # fold the previous pair's products while W accumulates
                    if cp == 0 and p > 0 and INTERLEAVE_FOLDS:
                        emit_folds()
                p1 = prp.tile([128, KL], BF16, tag="p1")
                p2 = prp.tile([128, KL], BF16, tag="p2")
                nc.vector.tensor_tensor(p1[:], W[:, :KL], uat[p][:],
                                        op=Alu.mult)
                nc.vector.tensor_tensor(p2[:], W[:, :KL], ubt[p][:],
                                        op=Alu.mult)
                prods.append((p1, p2))
            while fold_p[0] < NPAIR:
                emit_folds()

            # ---- residual: d = [kr;ki] - [kdr;kdi], sums ----
            d = rsp.tile([2 * CST, KL], F32, tag="d")
            sqb = rsp.tile([2 * CST, KL], BF16, tag="sqb")
            ssum = fpool.tile([CST, 1024], F32, tag="ssum")
            t2 = rsp.tile([CST, KL], F32, tag="t2")
            nc.vector.tensor_tensor(d[:], kri[:, :KL], kdri[:],
                                    op=Alu.subtract)
            sqf = rsp.tile([2 * CST, KL], F32, tag="sqf")
            nc.vector.scalar_tensor_tensor(sqf[:], d[:], 0.0, d[:],
                                           op0=Alu.bypass, op1=Alu.mult,
                                           accum_out=parts[:, 1:2])
            nc.vector.tensor_scalar(sqb[:], sqf[:], 0.0, None, op0=Alu.add)
            for (js, je) in JS:
                nc.tensor.matmul(ssum[:, js:je], sgn[0:2 * CST, 2, :],
                                 sqb[:, js:je], start=True, stop=True)
            nc.scalar.activation(t2[:], ssum[:, :KL], Sqrt,
                                 accum_out=parts[0:CST, 0:1])

            nc.sync.dma_start(parts_d[:], parts[:])

    nc.compile()
    return nc


_NC_CACHE = {}


def _get_nc(kl):
    if kl not in _NC_CACHE:
        _NC_CACHE[kl] = build_kernel(kl)
    return _NC_CACHE[kl]


def _prep_weights(images_reconstructed, sensitivity_maps):
    f8 = mybir.dt.np(F8)
    img = np.asarray(images_reconstructed)
    smaps = np.asarray(sensitivity_maps)
    x = 0.5 * img[None, ...] * smaps[..., None, None]       # (C,X,Y,1,1,T)
    xw = x.reshape(C, N, T).transpose(1, 0, 2).reshape(N, CST)  # n = nx*96+ny
    # regroup: [nx1, nx0, ny1, ny0] -> [g=(nx1,ny1), m=(nx0,ny0)]
    xg = xw.reshape(3, 32, 2, 48, CST).transpose(0, 2, 1, 3, 4).reshape(G, M, CST)
    xr = xg.real.astype(np.float32)
    xi = xg.imag.astype(np.float32)
    # w[pair, m0, cp, i, :]: DoubleRow weights, m = 128*(2*cp+i) + m0;
    # columns pack both groups of the pair: [xr_e|xi_e|xr_o|xi_o]
    w1 = np.empty((NPAIR, 128, CP, 2, 128), np.float32)
    w2 = np.empty((NPAIR, 128, CP, 2, 128), np.float32)
    for p in range(NPAIR):
        for gi, g in enumerate((2 * p, 2 * p + 1)):
            o = 64 * gi
            for ch in range(MCH):
                cp, half = divmod(ch, 2)
                sl = slice(128 * ch, 128 * (ch + 1))
                w1[p, :, cp, half, o:o + 32] = xr[g, sl]
                w1[p, :, cp, half, o + 32:o + 64] = xi[g, sl]
                w2[p, :, cp, half, o:o + 32] = -xi[g, sl]
                w2[p, :, cp, half, o + 32:o + 64] = xr[g, sl]
    return np.ascontiguousarray(w1.astype(f8)), np.ascontiguousarray(w2.astype(f8))


def make_in_maps(images_reconstructed, kspace_trajectory, kspace_data,
                 kspace_mask, sensitivity_maps, KL):
    f8 = mybir.dt.np(F8)
    KP = KL * NCORES
    traj = np.asarray(kspace_trajectory).astype(np.float32)
    kdata = np.asarray(kspace_data)
    mask = np.asarray(kspace_mask).astype(np.float32).reshape(K)

    w1, w2 = _prep_weights(images_reconstructed, sensitivity_maps)

    # gather kept columns, zero-pad to KP
    idx = np.flatnonzero(mask > 0)
    cnt = idx.size
    assert cnt <= KP, f"mask count {cnt} exceeds padded K {KP}"
    txg = np.zeros(KP, np.float64)
    tyg = np.zeros(KP, np.float64)
    txg[:cnt] = traj[0][idx]
    tyg[:cnt] = traj[1][idx]

    # V twiddle table (host, fp64 phase -> fp8): m = nx0*48 + ny0
    mm = np.arange(M)
    vx = (mm // 48 - 48).astype(np.float64)
    vy = (mm % 48 - 48).astype(np.float64)
    phs_v = vx[:, None] * txg[None, :] + vy[:, None] * tyg[None, :]  # (M, KP)
    vrf = np.cos(2 * np.pi * phs_v).astype(np.float32).astype(f8)
    vif = (-np.sin(2 * np.pi * phs_v)).astype(np.float32).astype(f8)
    # device layout [CP, 128, 2, KL-slice]; member chunk = 2*cp + i
    vr = vrf.reshape(CP, 2, 128, KP).transpose(0, 2, 1, 3)
    vi = vif.reshape(CP, 2, 128, KP).transpose(0, 2, 1, 3)

    # U twiddles with keep-mask, replicated f16 packs
    g_idx = np.arange(G)
    phs_u = ((32 * (g_idx // 2))[:, None] * txg[None, :]
             + (48 * (g_idx % 2))[:, None] * tyg[None, :])
    ur = np.cos(2 * np.pi * phs_u)
    ui = -np.sin(2 * np.pi * phs_u)
    keep = np.zeros(KP, np.float64)
    keep[:cnt] = 1.0
    ur *= keep[None, :]
    ui *= keep[None, :]
    ua = np.empty((NPAIR, 128, KP), np.float16)
    ub = np.empty((NPAIR, 128, KP), np.float16)
    for p in range(NPAIR):
        ua[p, 0:32] = ur[2 * p]
        ua[p, 32:64] = ui[2 * p]
        ua[p, 64:96] = ur[2 * p + 1]
        ua[p, 96:128] = ui[2 * p + 1]
        ub[p, 0:32] = ui[2 * p]
        ub[p, 32:64] = ur[2 * p]
        ub[p, 64:96] = ui[2 * p + 1]
        ub[p, 96:128] = ur[2 * p + 1]

    # sign matrices: fold the 4 blocks of P1/P2 (kr needs +,-,+,-; ki all +)
    # and the ones-fold pairing dr^2+di^2 (col 2)
    sgn = np.zeros((128, 3, CST), np.float32)
    for j in range(4):
        s = 1.0 if j % 2 == 0 else -1.0
        for c in range(CST):
            sgn[32 * j + c, 0, c] = s
            sgn[32 * j + c, 1, c] = 1.0
    for j in range(2):
        for c in range(CST):
            sgn[32 * j + c, 2, c] = 1.0
    sgn = sgn.astype(mybir.dt.np(BF16))

    # kdata at kept columns (mask=1 there); (K, CST) with c = coil*T + t
    kdm = kdata.reshape(C, K, T).transpose(1, 0, 2).reshape(K, CST)
    kg = np.zeros((KP, CST), np.complex64)
    kg[:cnt] = kdm[idx]

    in_maps = []
    for i in range(NCORES):
        ksl = slice(i * KL, (i + 1) * KL)
        kdri = np.concatenate([kg.real[ksl].T, kg.imag[ksl].T], axis=0)
        in_maps.append({
            "w1": w1, "w2": w2,
            "vr": np.ascontiguousarray(vr[:, :, :, ksl]),
            "vi": np.ascontiguousarray(vi[:, :, :, ksl]),
            "ua": np.ascontiguousarray(ua[:, :, ksl]),
            "ub": np.ascontiguousarray(ub[:, :, ksl]),
            "kdri": np.ascontiguousarray(kdri.astype(np.float32)),
            "sgn": sgn,
        })

    # host |a| sums (input-only, O(K))
    am = np.abs(kdm[idx]).astype(np.float64)
    sa1 = am.sum()
    sa2 = (am * am).sum()
    return in_maps, sa1, sa2


def combine(parts_list, sa1, sa2):
    tot0 = 0.0
    tot1 = 0.0
    for p in parts_list:
        p = p.astype(np.float64)
        tot0 += p[0:CST, 0].sum()
        tot1 += p[:, 1].sum()
    loss = W1 * (tot0 / sa1) + W2 * math.sqrt(tot1 / sa2)
    return np.asarray(loss, dtype=np.float32)


def kernel(images_reconstructed, kspace_trajectory, kspace_data,
           kspace_mask, sensitivity_maps, _trace=False):
    mask = np.asarray(kspace_mask).astype(np.float32).reshape(K)
    cnt = int((mask > 0).sum())
    KL = KL_PRIMARY if cnt <= KL_PRIMARY * NCORES else KL_FULL
    nc = _get_nc(KL)
    in_maps, sa1, sa2 = make_in_maps(images_reconstructed, kspace_trajectory,
                                     kspace_data, kspace_mask,
                                     sensitivity_maps, KL)
    res = run_bass_kernel_spmd(nc, in_maps, core_ids=list(range(NCORES)),
                               trace=_trace)
    out = combine([res.results[i]["parts"] for i in range(NCORES)], sa1, sa2)
    if _trace:
        return out, res
    return out


# revision 28
# speedup vs baseline: 1.9927x; 1.9927x over previous
"""Trainium2 Bass kernel for nn_KSpaceLoss: exact type-2 NUFFT k-space loss.

loss = 0.1 * (sum|d| / sum|a|) + 0.1 * sqrt(sum d^2 / sum a^2)
  d = (E @ x) * mask - kdata * mask,  a = kdata * mask
  E[k, n] = exp(-2j*pi * traj[:, k] . r[:, n])   (K=8192, N=96*96)

Strategy:
  * K axis: masked-out samples contribute 0 to both sums -> gather kept
    columns on host, pad to KP = 8*KL, shard over 8 cores (KL each).
  * Phase factorization: n=(nx,ny), nx=32*nx1+nx0, ny=48*ny1+ny0 gives
    E[n,k] = U[g,k] * V[m,k] with g=(nx1,ny1) in 6 groups and
    m=(nx0,ny0) in 1536 members. V ([1536,K] twiddle table, ~16% of E)
    and U ([6,K]) are host-precomputed; V is shipped as fp8e4.
  * ksp[k,c] = sum_g U[g,k] * W_g[k,c],  W_g = V^T @ x_g  as fp8e4
    DoubleRow matmuls (2 m-chunks per matmul, 0.5 cyc/row). Weights pack
    both groups of a pair into 128 columns [xr_e|xi_e|xr_o|xi_o]: stream
    Vr against that, Vi against [-xi_e|xr_e|-xi_o|xr_o], accumulating
    [Wre_e|Wim_e|Wre_o|Wim_o] in PSUM over all 12 member-chunks.
  * U applied per pair with two elementwise products (W * U-pack -> bf16)
    on DVE; each product is folded straight into kr/ki PSUM accumulators
    with +-1 sign matmuls on the PE (emitted one pair behind the DR
    stream so the PE never stalls). Zero-padding of the gathered K axis
    is folded into the U-packs (U=0 there -> ksp=0, kdata=0 -> d=0).
  * Residual: combined [64,KL] d = [kr;ki] - [kdr;kdi], one fused
    square+reduce (sum d^2), a ones-fold matmul for dr^2+di^2, and one
    Sqrt activation with accumulation (sum |d|). |a| sums are computed
    on host (O(K), input-only). Host does the final scalar combine.
"""

import math

import numpy as np

import concourse.bacc as bacc
import concourse.tile as tile
from concourse import mybir
from concourse.bass_utils import run_bass_kernel_spmd

X, Y = 96, 96
C, T = 8, 4
K = 8192
N = X * Y
NCORES = 8
CST = C * T               # 32
G = 6                     # groups: nx1 in [0,3), ny1 in [0,2)
M = 1536                  # members: nx0 in [0,32), ny0 in [0,48)
MCH = M // 128            # 12 member chunks
CP = MCH // 2             # 6 DoubleRow chunk-pairs
NPAIR = G // 2            # 3 group pairs
W1, W2 = 0.1, 0.1

F32 = mybir.dt.float32
F16 = mybir.dt.float16
BF16 = mybir.dt.bfloat16
F8 = mybir.dt.float8e4

KL_PRIMARY = 640          # per-core columns; covers mask count <= 5120
KL_FULL = 1024            # fallback: all 8192 columns fit


def _bank_slices(kl):
    out, j = [], 0
    while j < kl:
        je = min(j + 512, kl)
        out.append((j, je))
        j = je
    return out


def build_kernel(KL):
    nc = bacc.Bacc("TRN2", target_bir_lowering=False, debug=False,
                   num_devices=NCORES)

    w1_d = nc.dram_tensor("w1", [NPAIR, 128, CP, 2, 128], F8,
                          kind="ExternalInput").ap()
    w2_d = nc.dram_tensor("w2", [NPAIR, 128, CP, 2, 128], F8,
                          kind="ExternalInput").ap()
    vr_d = nc.dram_tensor("vr", [CP, 128, 2, KL], F8, kind="ExternalInput").ap()
    vi_d = nc.dram_tensor("vi", [CP, 128, 2, KL], F8, kind="ExternalInput").ap()
    ua_d = nc.dram_tensor("ua", [NPAIR, 128, KL], F16, kind="ExternalInput").ap()
    ub_d = nc.dram_tensor("ub", [NPAIR, 128, KL], F16, kind="ExternalInput").ap()
    kdri_d = nc.dram_tensor("kdri", [2 * CST, KL], F32, kind="ExternalInput").ap()
    sgn_d = nc.dram_tensor("sgn", [128, 3, CST], BF16, kind="ExternalInput").ap()
    parts_d = nc.dram_tensor("parts", [2 * CST, 2], F32, kind="ExternalOutput").ap()

    Sqrt = mybir.ActivationFunctionType.Sqrt
    Alu = mybir.AluOpType
    DR = mybir.MatmulPerfMode.DoubleRow
    JS = _bank_slices(KL)

    DMA_SPLIT = 1
    INTERLEAVE_FOLDS = False

    def dma4(dst, src):
        # split a [128, ...] load into partition slices -> parallel queues
        if DMA_SPLIT == 1:
            nc.sync.dma_start(dst[:], src[:])
            return
        step = 128 // DMA_SPLIT
        for q in range(DMA_SPLIT):
            sl = slice(step * q, step * (q + 1))
            nc.sync.dma_start(dst[sl], src[sl])

    with tile.TileContext(nc) as tc:
        with (
            tc.tile_pool(name="const", bufs=1) as cpool,
            tc.tile_pool(name="wacc", bufs=2, space="PSUM") as wpool,
            tc.tile_pool(name="fin", bufs=1, space="PSUM") as fpool,
            tc.tile_pool(name="prod", bufs=3) as prp,
            tc.tile_pool(name="resid", bufs=1) as rsp,
        ):
            # ---- constant loads, priority-ordered and queue-spread ----
            w1t = [cpool.tile([128, CP, 2, 128], F8, tag=f"w1_{p}", name=f"w1t{p}")
                   for p in range(NPAIR)]
            w2t = [cpool.tile([128, CP, 2, 128], F8, tag=f"w2_{p}", name=f"w2t{p}")
                   for p in range(NPAIR)]
            vr8 = cpool.tile([128, CP, 2, KL], F8, tag="vr8")
            vi8 = cpool.tile([128, CP, 2, KL], F8, tag="vi8")
            uat = [cpool.tile([128, KL], F16, tag=f"ua_{p}", name=f"uat{p}")
                   for p in range(NPAIR)]
            ubt = [cpool.tile([128, KL], F16, tag=f"ub_{p}", name=f"ubt{p}")
                   for p in range(NPAIR)]
            sgn = cpool.tile([128, 3, CST], BF16, tag="sgn")
            kdri = cpool.tile([2 * CST, KL], F32, tag="kdri")

            dma4(w1t[0][:], w1_d[0])
            dma4(vr8[:, 0], vr_d[0])
            dma4(w2t[0][:], w2_d[0])
            dma4(vi8[:, 0], vi_d[0])
            for cp in range(1, CP):
                dma4(vr8[:, cp], vr_d[cp])
                dma4(vi8[:, cp], vi_d[cp])
            nc.sync.dma_start(sgn[:], sgn_d[:])
            for p in range(1, NPAIR):
                dma4(w1t[p][:], w1_d[p])
                dma4(w2t[p][:], w2_d[p])
            for p in range(NPAIR):
                dma4(uat[p][:], ua_d[p])
                dma4(ubt[p][:], ub_d[p])
            nc.sync.dma_start(kdri[:], kdri_d[:])

            parts = rsp.tile([2 * CST, 2], F32, tag="parts")
            nc.vector.memset(parts[:], 0.0)

            # ---- per pair: PSUM W accumulation -> U products -> folds ----
            kri = fpool.tile([2 * CST, 1024], F32, tag="kri")
            prods = []          # (p1, p2) awaiting fold
            fold_p = [0]

            def emit_folds():
                p = fold_p[0]
                p1, p2 = prods[p]
                for (js, je) in JS:
                    nc.tensor.matmul(kri[0:CST, js:je], sgn[:, 0, :],
                                     p1[:, js:je],
                                     start=(p == 0), stop=(p == NPAIR - 1))
                for (js, je) in JS:
                    nc.tensor.matmul(kri[CST:2 * CST, js:je], sgn[:, 1, :],
                                     p2[:, js:je],
                                     start=(p == 0), stop=(p == NPAIR - 1))
                fold_p[0] += 1

            for p in range(NPAIR):
                W = wpool.tile([128, 1024], F32, tag="W")
                for cp in range(CP):
                    for (js, je) in JS:
                        nc.tensor.matmul(W[:, js:je], w1t[p][:, cp],
                                         vr8[:, cp, :, js:je],
                                         perf_mode=DR,
                                         start=(cp == 0), stop=False)
                    for (js, je) in JS:
                        nc.tensor.matmul(W[:, js:je], w2t[p][:, cp],
                                         vi8[:, cp, :, js:je],
                                         perf_mode=DR,
                                         start=False, stop=(cp == CP - 1))
                    # fold the previous pair's products while W accumulates
                    if cp == 0 and p > 0 and INTERLEAVE_FOLDS:
                        emit_folds()
                p1 = prp.tile([128, KL], BF16, tag="p1")
                p2 = prp.tile([128, KL], BF16, tag="p2")
                nc.vector.tensor_tensor(p1[:], W[:, :KL], uat[p][:],
                                        op=Alu.mult)
                nc.vector.tensor_tensor(p2[:], W[:, :KL], ubt[p][:],
                                        op=Alu.mult)
                prods.append((p1, p2))
            while fold_p[0] < NPAIR:
                emit_folds()

            # ---- residual: d = [kr;ki] - [kdr;kdi], sums ----
            d = rsp.tile([2 * CST, KL], F32, tag="d")
            sqb = rsp.tile([2 * CST, KL], BF16, tag="sqb")
            ssum = fpool.tile([CST, 1024], F32, tag="ssum")
            t2 = rsp.tile([CST, KL], F32, tag="t2")
            nc.vector.tensor_tensor(d[:], kri[:, :KL], kdri[:],
                                    op=Alu.subtract)
            sqf = rsp.tile([2 * CST, KL], F32, tag="sqf")
            nc.vector.scalar_tensor_tensor(sqf[:], d[:], 0.0, d[:],
                                           op0=Alu.bypass, op1=Alu.mult,
                                           accum_out=parts[:, 1:2])
            nc.vector.tensor_scalar(sqb[:], sqf[:], 0.0, None, op0=Alu.add)
            for (js, je) in JS:
                nc.tensor.matmul(ssum[:, js:je], sgn[0:2 * CST, 2, :],
                                 sqb[:, js:je], start=True, stop=True)
            nc.scalar.activation(t2[:], ssum[:, :KL], Sqrt,
                                 accum_out=parts[0:CST, 0:1])

            nc.sync.dma_start(parts_d[:], parts[:])

    nc.compile()
    return nc


_NC_CACHE = {}


def _get_nc(kl):
    if kl not in _NC_CACHE:
        _NC_CACHE[kl] = build_kernel(kl)
    return _NC_CACHE[kl]


def _prep_weights(images_reconstructed, sensitivity_maps):
    f8 = mybir.dt.np(F8)
    img = np.asarray(images_reconstructed)
    smaps = np.asarray(sensitivity_maps)
    x = 0.5 * img[None, ...] * smaps[..., None, None]       # (C,X,Y,1,1,T)
    xw = x.reshape(C, N, T).transpose(1, 0, 2).reshape(N, CST)  # n = nx*96+ny
    # regroup: [nx1, nx0, ny1, ny0] -> [g=(nx1,ny1), m=(nx0,ny0)]
    xg = xw.reshape(3, 32, 2, 48, CST).transpose(0, 2, 1, 3, 4).reshape(G, M, CST)
    xr = xg.real.astype(np.float32)
    xi = xg.imag.astype(np.float32)
    # w[pair, m0, cp, i, :]: DoubleRow weights, m = 128*(2*cp+i) + m0;
    # columns pack both groups of the pair: [xr_e|xi_e|xr_o|xi_o]
    w1 = np.empty((NPAIR, 128, CP, 2, 128), np.float32)
    w2 = np.empty((NPAIR, 128, CP, 2, 128), np.float32)
    for p in range(NPAIR):
        for gi, g in enumerate((2 * p, 2 * p + 1)):
            o = 64 * gi
            for ch in range(MCH):
                cp, half = divmod(ch, 2)
                sl = slice(128 * ch, 128 * (ch + 1))
                w1[p, :, cp, half, o:o + 32] = xr[g, sl]
                w1[p, :, cp, half, o + 32:o + 64] = xi[g, sl]
                w2[p, :, cp, half, o:o + 32] = -xi[g, sl]
                w2[p, :, cp, half, o + 32:o + 64] = xr[g, sl]
    return np.ascontiguousarray(w1.astype(f8)), np.ascontiguousarray(w2.astype(f8))


def make_in_maps(images_reconstructed, kspace_trajectory, kspace_data,
                 kspace_mask, sensitivity_maps, KL):
    f8 = mybir.dt.np(F8)
    KP = KL * NCORES
    traj = np.asarray(kspace_trajectory).astype(np.float32)
    kdata = np.asarray(kspace_data)
    mask = np.asarray(kspace_mask).astype(np.float32).reshape(K)

    w1, w2 = _prep_weights(images_reconstructed, sensitivity_maps)

    # gather kept columns, zero-pad to KP
    idx = np.flatnonzero(mask > 0)
    cnt = idx.size
    assert cnt <= KP, f"mask count {cnt} exceeds padded K {KP}"
    txg = np.zeros(KP, np.float64)
    tyg = np.zeros(KP, np.float64)
    txg[:cnt] = traj[0][idx]
    tyg[:cnt] = traj[1][idx]

    # V twiddle table (host, fp64 phase -> fp8): m = nx0*48 + ny0
    mm = np.arange(M)
    vx = (mm // 48 - 48).astype(np.float64)
    vy = (mm % 48 - 48).astype(np.float64)
    phs_v = vx[:, None] * txg[None, :] + vy[:, None] * tyg[None, :]  # (M, KP)
    vrf = np.cos(2 * np.pi * phs_v).astype(np.float32).astype(f8)
    vif = (-np.sin(2 * np.pi * phs_v)).astype(np.float32).astype(f8)
    # device layout [CP, 128, 2, KL-slice]; member chunk = 2*cp + i
    vr = vrf.reshape(CP, 2, 128, KP).transpose(0, 2, 1, 3)
    vi = vif.reshape(CP, 2, 128, KP).transpose(0, 2, 1, 3)

    # U twiddles with keep-mask, replicated f16 packs
    g_idx = np.arange(G)
    phs_u = ((32 * (g_idx // 2))[:, None] * txg[None, :]
             + (48 * (g_idx % 2))[:, None] * tyg[None, :])
    ur = np.cos(2 * np.pi * phs_u)
    ui = -np.sin(2 * np.pi * phs_u)
    keep = np.zeros(KP, np.float64)
    keep[:cnt] = 1.0
    ur *= keep[None, :]
    ui *= keep[None, :]
    ua = np.empty((NPAIR, 128, KP), np.float16)
    ub = np.empty((NPAIR, 128, KP), np.float16)
    for p in range(NPAIR):
        ua[p, 0:32] = ur[2 * p]
        ua[p, 32:64] = ui[2 * p]
        ua[p, 64:96] = ur[2 * p + 1]
        ua[p, 96:128] = ui[2 * p + 1]
        ub[p, 0:32] = ui[2 * p]
        ub[p, 32:64] = ur[2 * p]
        ub[p, 64:96] = ui[2 * p + 1]
        ub[p, 96:128] = ur[2 * p + 1]

    # sign matrices: fold the 4 blocks of P1/P2 (kr needs +,-,+,-; ki all +)
    # and the ones-fold pairing dr^2+di^2 (col 2)
    sgn = np.zeros((128, 3, CST), np.float32)
    for j in range(4):
        s = 1.0 if j % 2 == 0 else -1.0
        for c in range(CST):
            sgn[32 * j + c, 0, c] = s
            sgn[32 * j + c, 1, c] = 1.0
    for j in range(2):
        for c in range(CST):
            sgn[32 * j + c, 2, c] = 1.0
    sgn = sgn.astype(mybir.dt.np(BF16))

    # kdata at kept columns (mask=1 there); (K, CST) with c = coil*T + t
    kdm = kdata.reshape(C, K, T).transpose(1, 0, 2).reshape(K, CST)
    kg = np.zeros((KP, CST), np.complex64)
    kg[:cnt] = kdm[idx]

    in_maps = []
    for i in range(NCORES):
        ksl = slice(i * KL, (i + 1) * KL)
        kdri = np.concatenate([kg.real[ksl].T, kg.imag[ksl].T], axis=0)
        in_maps.append({
            "w1": w1, "w2": w2,
            "vr": np.ascontiguousarray(vr[:, :, :, ksl]),
            "vi": np.ascontiguousarray(vi[:, :, :, ksl]),
            "ua": np.ascontiguousarray(ua[:, :, ksl]),
            "ub": np.ascontiguousarray(ub[:, :, ksl]),
            "kdri": np.ascontiguousarray(kdri.astype(np.float32)),
            "sgn": sgn,
        })

    # host |a| sums (input-only, O(K))
    am = np.abs(kdm[idx]).astype(np.float64)
    sa1 = am.sum()
    sa2 = (am * am).sum()
    return in_maps, sa1, sa2


def combine(parts_list, sa1, sa2):
    tot0 = 0.0
    tot1 = 0.0
    for p in parts_list:
        p = p.astype(np.float64)
        tot0 += p[0:CST, 0].sum()
        tot1 += p[:, 1].sum()
    loss = W1 * (tot0 / sa1) + W2 * math.sqrt(tot1 / sa2)
    return np.asarray(loss, dtype=np.float32)


def kernel(images_reconstructed, kspace_trajectory, kspace_data,
           kspace_mask, sensitivity_maps, _trace=False):
    mask = np.asarray(kspace_mask).astype(np.float32).reshape(K)
    cnt = int((mask > 0).sum())
    KL = KL_PRIMARY if cnt <= KL_PRIMARY * NCORES else KL_FULL
    nc = _get_nc(KL)
    in_maps, sa1, sa2 = make_in_maps(images_reconstructed, kspace_trajectory,
                                     kspace_data, kspace_mask,
                                     sensitivity_maps, KL)
    res = run_bass_kernel_spmd(nc, in_maps, core_ids=list(range(NCORES)),
                               trace=_trace)
    out = combine([res.results[i]["parts"] for i in range(NCORES)], sa1, sa2)
    if _trace:
        return out, res
    return out
